# revision 9
# baseline (speedup 1.0000x reference)
"""Trainium2 Bass kernel for nn_BeamSearchDecoder (B=32, K=3, H=512, E=256,
V=32000, 32 decode steps), SPMD over 8 NeuronCores.

Key observation (verified against the reference): all K=3 beams start from an
identical state (h0 repeated, same START token, zero scores) and jax.lax.top_k
breaks ties by lower index, so the beam search is exactly greedy decoding with
every beam identical at every step (bitwise).  The kernel therefore runs a
greedy GRU decoder over 32 batch rows and the host replicates beams / builds
the one-hot output.

Distribution: the output projection W_out (32000x512 fp32 = 65.5 MB) is
sharded over the 8 cores by vocab (4000 rows each, SBUF-resident).  Each step
every core computes its logits shard, a local top-1 (+ sum-exp for the
log-softmax normalizer), and the 8 per-core candidates are combined with a
small AllGather; every core then picks the global argmax as the next token.
Per-step per-core candidate data (max logit, global argmax index, sumexp) is
also written to an output so the host can rebuild sequences and scores.

The GRU input projection is folded into a host-precomputed table
  gi_table[v] = relu(emb[v]) @ W_ih.T + b_ih + [b_hh_r, b_hh_z, 0]
gathered per step with an indirect DMA (the n-gate b_hh part must stay inside
the r*(...) term, so it is added separately on device).
"""

import sys

import numpy as np

for _p in ("/opt/trn_rl_repo", "/root/.axon_site/_ro/trn_rl_repo"):
    if _p not in sys.path:
        sys.path.append(_p)

import concourse.bass as bass
import concourse.mybir as mybir
import concourse.tile as tile
from concourse import bacc
from concourse.bass_utils import run_bass_kernel_spmd
from concourse.masks import make_identity

F32 = mybir.dt.float32
I32 = mybir.dt.int32
U32 = mybir.dt.uint32
AF = mybir.ActivationFunctionType
OP = mybir.AluOpType
AX = mybir.AxisListType

P = 128
NCORES = 8
B = 32          # batch
K = 3           # beams (degenerate/identical)
H = 512
E = 256
V = 32000
STEPS = 32
START = 1
VSH = V // NCORES          # 4000 vocab per core
QW = VSH // 4              # 1000 per quarter-partition-group
KSUB = H // P              # 4
BIG = 1.0e9


def _build_nc():
    nc = bacc.Bacc("TRN2", target_bir_lowering=False, debug=False,
                   num_devices=NCORES)

    gi_table = nc.dram_tensor("gi_table", [V, 3 * H], F32, kind="ExternalInput")
    w_hht = nc.dram_tensor("w_hht", [P, KSUB, 3 * H], F32, kind="ExternalInput")
    b_hhn = nc.dram_tensor("b_hhn", [B, H], F32, kind="ExternalInput")
    wo = nc.dram_tensor("wo", [P, 16, QW], F32, kind="ExternalInput")
    bo = nc.dram_tensor("bo", [P, QW], F32, kind="ExternalInput")
    h0t = nc.dram_tensor("h0t", [P, KSUB, B], F32, kind="ExternalInput")
    h0row = nc.dram_tensor("h0row", [B, H], F32, kind="ExternalInput")
    vocoff = nc.dram_tensor("vocoff", [P, 1], F32, kind="ExternalInput")

    steps_out = nc.dram_tensor("steps_out", [STEPS, 1, 3 * B], F32,
                               kind="ExternalOutput")
    h_out = nc.dram_tensor("h_out", [B, H], F32, kind="ExternalOutput")

    with tile.TileContext(nc) as tc:
        with tc.tile_pool(name="const", bufs=1) as cp, \
             tc.tile_pool(name="sb", bufs=2) as sb, \
             tc.tile_pool(name="ps", bufs=1, space="PSUM") as ps, \
             tc.tile_pool(name="dr", bufs=2, space="DRAM") as dr:

            # ---- resident constants ----
            w_hht_sb = cp.tile([P, KSUB, 3 * H], F32)
            nc.sync.dma_start(w_hht_sb[:], w_hht[:])
            wo_sb = cp.tile([P, 16, QW], F32)
            nc.sync.dma_start(wo_sb[:], wo[:])
            bo_sb = cp.tile([P, QW], F32)
            nc.sync.dma_start(bo_sb[:], bo[:])
            b_hhn_sb = cp.tile([B, H], F32)
            nc.sync.dma_start(b_hhn_sb[:], b_hhn[:])
            vocoff_sb = cp.tile([P, 1], F32)
            nc.sync.dma_start(vocoff_sb[:], vocoff[:])
            ident = cp.tile([P, P], F32)
            make_identity(nc, ident[:])
            big_sb = cp.tile([P, 1], F32)
            nc.vector.memset(big_sb[:], BIG)

            # ---- state ----
            hT = cp.tile([P, KSUB, B], F32, name="hT0")
            nc.sync.dma_start(hT[:], h0t[:])
            hrow = cp.tile([B, H], F32, name="hrow0")
            nc.sync.dma_start(hrow[:], h0row[:])
            tok = cp.tile([B, 1], I32, name="tok0")
            nc.vector.memset(tok[:], START)

            for t in range(STEPS):
                # 1. gather gi = gi_table[tok]  -> [B, 3H]
                gi = sb.tile([B, 3 * H], F32, tag="gi")
                nc.gpsimd.indirect_dma_start(
                    out=gi[:], out_offset=None,
                    in_=gi_table[:],
                    in_offset=bass.IndirectOffsetOnAxis(ap=tok[:, :1], axis=0),
                )

                # 2. gh matmuls: RZ [B,1024], HN [B,512]
                rz_ps = ps.tile([B, 2 * H], F32, tag="rz")
                hn_ps = ps.tile([B, H], F32, tag="hn")
                for ch in range(2):
                    for k in range(KSUB):
                        nc.tensor.matmul(
                            rz_ps[:, ch * H:(ch + 1) * H],
                            lhsT=hT[:, k, :],
                            rhs=w_hht_sb[:, k, ch * H:(ch + 1) * H],
                            start=(k == 0), stop=(k == KSUB - 1))
                for k in range(KSUB):
                    nc.tensor.matmul(
                        hn_ps[:],
                        lhsT=hT[:, k, :],
                        rhs=w_hht_sb[:, k, 2 * H:3 * H],
                        start=(k == 0), stop=(k == KSUB - 1))

                # 3. r,z = sigmoid(gh_rz + gi_rz)
                rzs = sb.tile([B, 2 * H], F32, tag="rzs")
                nc.vector.tensor_add(rzs[:], rz_ps[:], gi[:, :2 * H])
                sig = sb.tile([B, 2 * H], F32, tag="sig")
                nc.scalar.activation(sig[:], rzs[:], AF.Sigmoid)

                # 4. n = tanh(gi_n + r * (gh_n + b_hh_n))
                hn2 = sb.tile([B, H], F32, tag="hn2")
                nc.vector.tensor_add(hn2[:], hn_ps[:], b_hhn_sb[:])
                nc.vector.tensor_mul(hn2[:], hn2[:], sig[:, :H])
                nc.vector.tensor_add(hn2[:], hn2[:], gi[:, 2 * H:3 * H])
                n_sb = sb.tile([B, H], F32, tag="n")
                nc.scalar.activation(n_sb[:], hn2[:], AF.Tanh)

                # 5. h_new = n + z * (h - n)
                hrow_new = sb.tile([B, H], F32, tag="hrow")
                nc.vector.tensor_sub(hrow_new[:], hrow[:], n_sb[:])
                nc.vector.tensor_mul(hrow_new[:], hrow_new[:], sig[:, H:2 * H])
                nc.vector.tensor_add(hrow_new[:], hrow_new[:], n_sb[:])

                # 6. transpose h_new -> hT_new [128, 4, B]
                ht_ps = ps.tile([P, KSUB * B], F32, tag="htp")
                for k in range(KSUB):
                    nc.tensor.transpose(ht_ps[:, k * B:(k + 1) * B],
                                        hrow_new[:, k * P:(k + 1) * P],
                                        ident[:B, :B])
                hT_new = sb.tile([P, KSUB, B], F32, tag="hT")
                nc.vector.tensor_copy(hT_new[:].rearrange("p k b -> p (k b)"),
                                      ht_ps[:])

                # 7. logits shard, quarter-packed: [128, 1000]
                #    partition p = b + 32q covers vocab [q*1000, (q+1)*1000)
                lg = ps.tile([P, 1024], F32, tag="lg")
                for q in range(4):
                    for c0, c1 in ((0, H), (H, QW)):
                        for k in range(KSUB):
                            nc.tensor.matmul(
                                lg[q * B:(q + 1) * B, c0:c1],
                                lhsT=hT_new[:, k, :],
                                rhs=wo_sb[:, q * KSUB + k, c0:c1],
                                start=(k == 0), stop=(k == KSUB - 1),
                                tile_position=(0, q * B))
                nc.vector.tensor_add(lg[:, :QW], lg[:, :QW], bo_sb[:])

                # 8. local top1 + sumexp
                v8 = sb.tile([P, 8], F32, tag="v8")
                nc.vector.max(v8[:], lg[:, :QW])
                i8 = sb.tile([P, 8], U32, tag="i8")
                nc.vector.max_index(i8[:], v8[:], lg[:, :QW])
                esc = sb.tile([P, QW], F32, tag="esc")
                se = sb.tile([P, 1], F32, tag="se")
                nc.scalar.activation(esc[:], lg[:, :QW], AF.Exp,
                                     accum_out=se[:])

                # 9. per-partition candidate columns: val / global idx / sumexp
                gix = sb.tile([P, 1], F32, tag="gix")
                nc.vector.tensor_copy(gix[:], i8[:, 0:1])
                nc.vector.tensor_scalar(gix[:], gix[:],
                                        vocoff_sb[:, 0:1], None, op0=OP.add)

                # 10. transpose the three columns to one partition-0 row
                #     mrow[0, 128f + p] = field f of partition p  (p = b+32q)
                mrow_ps = ps.tile([1, 3 * P], F32, tag="mrow")
                nc.tensor.transpose(mrow_ps[:, 0:P], v8[:, 0:1], ident[:])
                nc.tensor.transpose(mrow_ps[:, P:2 * P], gix[:], ident[:])
                nc.tensor.transpose(mrow_ps[:, 2 * P:3 * P], se[:], ident[:])
                mrow = sb.tile([1, 3 * P], F32, tag="mrowsb")
                nc.vector.tensor_copy(mrow[:], mrow_ps[:])

                # quarter-merge -> per-batch row prow = [val | gidx | sumexp]
                val_v = mrow[0:1, 0:P].rearrange("o (q b) -> o b q", q=4)
                idx_v = mrow[0:1, P:2 * P].rearrange("o (q b) -> o b q", q=4)
                se_v = mrow[0:1, 2 * P:3 * P].rearrange("o (q b) -> o b q", q=4)
                prow = sb.tile([1, 3 * B], F32, tag="prow")
                nc.vector.tensor_reduce(out=prow[0:1, 0:B], in_=val_v,
                                        op=OP.max, axis=AX.X)
                eqq = sb.tile([1, B, 4], mybir.dt.uint8, tag="eqq")
                nc.vector.tensor_tensor(
                    eqq[:], val_v,
                    prow[0:1, 0:B, None].to_broadcast([1, B, 4]), OP.is_equal)
                isel = sb.tile([1, B, 4], F32, tag="isel")
                nc.vector.select(isel[:], eqq[:], idx_v,
                                 big_sb[0:1, 0:1, None].to_broadcast([1, B, 4]))
                nc.vector.tensor_reduce(out=prow[0:1, B:2 * B], in_=isel[:],
                                        op=OP.min, axis=AX.X)
                nc.vector.tensor_reduce(out=prow[0:1, 2 * B:3 * B], in_=se_v,
                                        op=OP.add, axis=AX.X)

                # own-core per-step record -> host
                nc.sync.dma_start(steps_out[t], prow[:])

                if t == STEPS - 1:
                    break   # host has everything it needs; no next token

                # 11. exchange candidates
                cc_in = dr.tile([1, 3 * B], F32, tag="ccin")
                cc_out = dr.tile([NCORES, 3 * B], F32, addr_space="Shared",
                                 tag="ccout")
                nc.sync.dma_start(cc_in[:], prow[:])
                nc.gpsimd.collective_compute(
                    "AllGather", OP.bypass,
                    replica_groups=[list(range(NCORES))],
                    ins=[cc_in[:].opt()], outs=[cc_out[:].opt()],
                )
                recv = sb.tile([1, NCORES * 3 * B], F32, tag="recv")
                nc.sync.dma_start(recv[:],
                                  cc_out[:].rearrange("c f -> (c f)")[None, :])

                # 12. global argmax -> next token (ties: lowest vocab index)
                rv = recv[0:1, :].rearrange("o (c f b) -> o b c f",
                                            c=NCORES, f=3)
                val_c = rv[:, :, :, 0]
                idx_c = rv[:, :, :, 1]
                vg = sb.tile([1, B], F32, tag="vg")
                nc.vector.tensor_reduce(out=vg[:], in_=val_c,
                                        op=OP.max, axis=AX.X)
                eq2 = sb.tile([1, B, NCORES], mybir.dt.uint8, tag="eq2")
                nc.vector.tensor_tensor(
                    eq2[:], val_c,
                    vg[0:1, :, None].to_broadcast([1, B, NCORES]),
                    OP.is_equal)
                tsel = sb.tile([1, B, NCORES], F32, tag="tsel")
                nc.vector.select(
                    tsel[:], eq2[:], idx_c,
                    big_sb[0:1, 0:1, None].to_broadcast([1, B, NCORES]))
                tokrow = sb.tile([1, B], F32, tag="tokrow")
                nc.vector.tensor_reduce(out=tokrow[:], in_=tsel[:],
                                        op=OP.min, axis=AX.X)
                tok_ps = ps.tile([B, 1], F32, tag="tokp")
                nc.tensor.transpose(tok_ps[:], tokrow[:], ident[:1, :1])
                tok_new = sb.tile([B, 1], I32, tag="tok")
                nc.vector.tensor_copy(tok_new[:], tok_ps[:])

                hT, hrow, tok = hT_new, hrow_new, tok_new

            nc.sync.dma_start(h_out[:], hrow_new[:])

    nc.compile()
    return nc


_NC_CACHE = None


def _get_nc():
    global _NC_CACHE
    if _NC_CACHE is None:
        _NC_CACHE = _build_nc()
    return _NC_CACHE


def _host_prep(encoder_hidden, emb, W_ih, W_hh, b_ih, b_hh, W_out, b_out):
    emb = np.asarray(emb, np.float32)
    W_ih = np.asarray(W_ih, np.float32)
    W_hh = np.asarray(W_hh, np.float32)
    b_ih = np.asarray(b_ih, np.float32)
    b_hh = np.asarray(b_hh, np.float32)
    W_out = np.asarray(W_out, np.float32)
    b_out = np.asarray(b_out, np.float32)
    h0 = np.asarray(encoder_hidden, np.float32)

    bias = b_ih.copy()
    bias[:2 * H] += b_hh[:2 * H]
    gi_table = np.maximum(emb, 0.0).astype(np.float32) @ W_ih.T + bias
    gi_table = np.ascontiguousarray(gi_table, np.float32)

    w_hht = np.ascontiguousarray(
        W_hh.T.reshape(KSUB, P, 3 * H).transpose(1, 0, 2), np.float32)
    b_hhn = np.ascontiguousarray(
        np.broadcast_to(b_hh[2 * H:], (B, H)), np.float32)
    h0t = np.ascontiguousarray(
        h0.T.reshape(KSUB, P, B).transpose(1, 0, 2), np.float32)

    common = {
        "gi_table": gi_table,
        "w_hht": w_hht,
        "b_hhn": b_hhn,
        "h0t": h0t,
        "h0row": np.ascontiguousarray(h0, np.float32),
    }
    in_maps = []
    for c in range(NCORES):
        wc = W_out[c * VSH:(c + 1) * VSH]                      # [4000, 512]
        wo = np.ascontiguousarray(
            wc.T.reshape(KSUB, P, 4, QW).transpose(1, 2, 0, 3)
            .reshape(P, 16, QW), np.float32)
        bc = b_out[c * VSH:(c + 1) * VSH].reshape(4, QW)
        bo = np.ascontiguousarray(np.repeat(bc, B, axis=0), np.float32)
        vocoff = (np.arange(P) // B * QW + c * VSH).astype(np.float32)[:, None]
        m = dict(common)
        m.update({"wo": wo, "bo": bo,
                  "vocoff": np.ascontiguousarray(vocoff)})
        in_maps.append(m)
    return in_maps


def _host_finish(results, h_final):
    """Rebuild (decoded, h, scores) from per-core per-step candidates."""
    pays = np.stack([r["steps_out"][:, 0] for r in results])  # [C, T, 3B]
    vals = pays[:, :, 0:B].astype(np.float64)              # [C, T, B]
    gidx = pays[:, :, B:2 * B].astype(np.int64)
    sume = pays[:, :, 2 * B:3 * B].astype(np.float64)

    vmax = vals.max(axis=0)                                # [T, B]
    tok = np.where(vals == vmax[None], gidx, np.int64(1 << 40)).min(axis=0)
    lse = np.log(sume.sum(axis=0))                         # [T, B]
    cum = (vmax - lse).sum(axis=0)                         # [B]

    seqs = np.full((B, STEPS + 1), START, np.int64)
    seqs[:, 1:] = tok.T                                    # greedy: old = id
    decoded = np.zeros((B, STEPS + 1, V), np.float32)
    bi = np.repeat(np.arange(B), STEPS + 1)
    ti = np.tile(np.arange(STEPS + 1), B)
    decoded[bi, ti, seqs.reshape(-1)] = 1.0

    h = np.repeat(h_final[:, None, :], K, axis=1).astype(np.float32)
    scores = np.repeat(cum.astype(np.float32)[:, None], K, axis=1)
    return decoded, h, scores


def kernel(encoder_outputs, encoder_hidden, emb, W_ih, W_hh, b_ih, b_hh,
           W_out, b_out):
    nc = _get_nc()
    in_maps = _host_prep(encoder_hidden, emb, W_ih, W_hh, b_ih, b_hh,
                         W_out, b_out)
    res = run_bass_kernel_spmd(nc, in_maps, core_ids=list(range(NCORES)),
                               trace=False)
    return _host_finish(res.results, res.results[0]["h_out"])


if __name__ == "__main__":
    # quick self-driven run with random inputs
    rng = np.random.default_rng(0)
    ins = {
        "encoder_outputs": rng.standard_normal((B, 64, H), np.float32),
        "encoder_hidden": rng.standard_normal((B, H), np.float32),
        "emb": (rng.standard_normal((V, E)) * 0.02).astype(np.float32),
        "W_ih": rng.uniform(-1 / 16, 1 / 16, (3 * H, E)).astype(np.float32),
        "W_hh": rng.uniform(-1 / 22.6, 1 / 22.6, (3 * H, H)).astype(np.float32),
        "b_ih": rng.uniform(-1 / 22.6, 1 / 22.6, (3 * H,)).astype(np.float32),
        "b_hh": rng.uniform(-1 / 22.6, 1 / 22.6, (3 * H,)).astype(np.float32),
        "W_out": rng.uniform(-1 / 22.6, 1 / 22.6, (V, H)).astype(np.float32),
        "b_out": rng.uniform(-1 / 22.6, 1 / 22.6, (V,)).astype(np.float32),
    }
    out = kernel(**ins)
    print([o.shape for o in out])
